# revision 37
# baseline (speedup 1.0000x reference)
"""Trainium2 Bass kernel for nn_MHInrAttn (sparse_attention, b=4 s=1024 f=1024 h=16).

Strategy (8 NeuronCores):
  - The reference uses a raw .reshape(b, h, s, d_h) with NO transpose, so head h's
    Q/K/V data comes from ROWS [64h, 64h+64) of the projected [s, f] matrix.
    Sharding 2 heads per core means each core only needs 128 rows of x per batch.
  - Per core: project Q/K/V for its 128 rows (all 4 batches), run attention for its
    2 heads x 4 batches in a "transposed" orientation (scores^T [k, q]), and produce
    a partial output projection (its heads' contribution through Wo rows).
  - Host: shard inputs, run SPMD on 8 cores, sum the 8 partials, transpose, add bo.

Device-side details:
  - Q^T/K^T are produced directly by W^T @ x^T matmuls (batches packed in the
    moving dim); DVE copies shuffle psum into the [d + 64*hp, q'] layout with the
    projection bias folded in via tensor_scalar_add (no PE transposes).
  - str_mat is host-transposed+masked (-40 fill), sent bf16; exp runs on Act as
    soon as each chunk's DMA lands (hoisted ahead of the projection phase).
  - softmax1: rowsums via ones-column matmuls into one psum bank (rows 0/32/64),
    1/rowsum broadcast via K=1 matmuls; eM is then normalized in place (bf16 DVE).
  - PV matmul carries an extra ones column in V for softmax2 row sums.
  - All matmul operands are bf16 (1 cycle/row); psum accumulation and the
    exp-argument chain stay fp32. Output partials are written bf16 and summed
    in fp32 on the host.
"""

import numpy as np

B, S, F, H, D = 4, 1024, 1024, 16, 64
NCORES = 8
HPC = H // NCORES  # heads per core
P = 128
NEG_FILL = -40.0
MM_DT = "bfloat16"
# psum partition row per (hp, h2) r1 accumulation chain; matmul output base
# partitions must be 0/32/64, so the 4th chain reuses row 0 (freed by the
# hp=0 reciprocal long before, in program order).
R1ROW = {(0, 0): 0, (0, 1): 32, (1, 0): 64, (1, 1): 0}

_CACHE = {}


def _build_nc(mm_dt_name="bfloat16", causal=True):
    from contextlib import ExitStack

    import concourse.bacc as bacc
    import concourse.tile as tile
    from concourse import mybir

    dt = mybir.dt
    f32 = dt.float32
    mmdt = getattr(dt, mm_dt_name)
    Exp = mybir.ActivationFunctionType.Exp
    Identity = mybir.ActivationFunctionType.Identity

    nc = bacc.Bacc("TRN2", target_bir_lowering=False, debug=False)

    xT_d = nc.dram_tensor("xT", [P, 8, 4 * P], mmdt, kind="ExternalInput").ap()
    str_d = nc.dram_tensor("strT", [B, HPC, S, S], mmdt, kind="ExternalInput").ap()
    wq_d = nc.dram_tensor("wq", [F, F], mmdt, kind="ExternalInput").ap()
    wk_d = nc.dram_tensor("wk", [F, F], mmdt, kind="ExternalInput").ap()
    wv_d = nc.dram_tensor("wv", [F, F], mmdt, kind="ExternalInput").ap()
    wo_d = nc.dram_tensor("wo", [P, F], mmdt, kind="ExternalInput").ap()
    bqkT_d = nc.dram_tensor("bqkT", [P, 32], f32, kind="ExternalInput").ap()
    bv_d = nc.dram_tensor("bv", [1, F], mmdt, kind="ExternalInput").ap()
    out_d = nc.dram_tensor("outT", [B, F, S], mmdt, kind="ExternalOutput").ap()

    with ExitStack() as ctx:
        ctx.enter_context(nc.allow_low_precision(
            reason="bf16 operands; all matmul accumulation stays in f32 psum"))
        tc = ctx.enter_context(tile.TileContext(nc))
        consts = ctx.enter_context(tc.tile_pool(name="consts", bufs=1))
        qtkt = ctx.enter_context(tc.tile_pool(name="qtkt", bufs=1))
        v2p = ctx.enter_context(tc.tile_pool(name="v2", bufs=1))
        outp = ctx.enter_context(tc.tile_pool(name="outp", bufs=1))
        wop = ctx.enter_context(tc.tile_pool(name="wop", bufs=1))
        emp = ctx.enter_context(tc.tile_pool(name="em", bufs=4))
        dramp = ctx.enter_context(tc.tile_pool(name="dram", bufs=1, space="DRAM"))

        ones_all = consts.tile([P, P], mmdt, tag="ones", name="ones")
        nc.vector.memset(ones_all, 1.0)
        bqkT_sb = consts.tile([P, 32], f32, tag="bqkT", name="bqkT")
        nc.sync.dma_start(out=bqkT_sb, in_=bqkT_d)
        bv_sb = consts.tile([1, F], mmdt, tag="bv", name="bv")
        nc.sync.dma_start(out=bv_sb, in_=bv_d)
        wo_sb = wop.tile([P, F], mmdt, tag="wo", name="wo")
        nc.sync.dma_start(out=wo_sb, in_=wo_d)

        QT_all = qtkt.tile([P, B * S], mmdt, tag="qt", name="qt")
        KT_all = qtkt.tile([P, B * S], mmdt, tag="kt", name="kt")
        OT, V2 = {}, {}
        for b in range(B):
            OT[b] = outp.tile([P, S], mmdt, tag=f"ot{b}", name=f"ot{b}")
            for hp in range(HPC):
                V2[b, hp] = v2p.tile([P, 8, P], mmdt, tag=f"v{b}{hp}", name=f"v{b}{hp}")

        # str_mat loads: DMA (SP, dep-free) + exp (Act), all four batches resident
        eMslots = [dict() for _ in range(B)]

        def load_dma(b):
            slot = eMslots[b]
            for hp in range(HPC):
                for j in range(8):
                    jl = 128 * j if causal else 0
                    t = emp.tile([P, S - jl], mmdt, tag=f"e{hp}{j}", name=f"e{hp}{j}")
                    slot[hp, j] = t
                    nc.sync.dma_start(out=t, in_=str_d[b, hp, 128 * j:128 * (j + 1), jl:])

        def load_exp(b, hps=(0, 1)):
            slot = eMslots[b]
            for hp in hps:
                for j in range(8):
                    nc.scalar.activation(slot[hp, j], slot[hp, j], Exp)

        # ---------- phase 1: projections + layout shuffles ----------
        with tc.tile_pool(name="xt", bufs=1) as xtp, \
                tc.tile_pool(name="wpool", bufs=1) as wp, \
                tc.tile_pool(name="qkvc", bufs=1) as qkvcp, \
                tc.tile_pool(name="pj", bufs=4, space="PSUM") as ppool:
            xt_all = xtp.tile([P, 8, 4 * P], mmdt, tag="xt", name="xt")
            nc.sync.dma_start(out=xt_all, in_=xT_d)

            # all weight DMAs up front (distinct tags; no dependent DMA may
            # precede a dep-free one on the in-order SP queue)
            wts = {}
            for t_i, w_d in enumerate((wq_d, wv_d, wk_d)):
                wts[t_i] = []
                for i in range(8):
                    w_tile = wp.tile([P, F], mmdt, tag=f"w{t_i}{i}", name=f"w{t_i}{i}")
                    nc.sync.dma_start(out=w_tile, in_=w_d[i * P:(i + 1) * P, :])
                    wts[t_i].append(w_tile)
            load_dma(0)
            load_dma(1)

            def proj_qk(t_i, dstT):
                wt = wts[0 if t_i == 0 else 2]
                for i in range(8):
                    ps = ppool.tile([P, 512], f32, tag="pj", name="pj")
                    for kc in range(8):
                        nc.tensor.matmul(ps, wt[kc][:, 128 * i:128 * (i + 1)],
                                         xt_all[:, kc, :],
                                         start=(kc == 0), stop=(kc == 7))
                    ps4 = ps.rearrange("p (b h r) -> p b h r", b=4, h=2)
                    for ch in range(2):
                        cb = 2 * i + ch
                        for hp in range(2):
                            src = ps4[64 * ch:64 * ch + 64, :, hp, :]
                            dst = dstT[64 * hp:64 * hp + 64, :].rearrange(
                                "p (b r c) -> p b r c", b=4, c=16)[:, :, :, cb]
                            bias_ap = bqkT_sb[64 * hp:64 * hp + 64,
                                              16 * t_i + cb:16 * t_i + cb + 1]
                            nc.vector.tensor_scalar_add(dst, src, bias_ap)

            proj_qk(0, QT_all)
            load_exp(0)

            # V: x @ Wv orientation (rows on partitions), shuffle through DRAM
            wt = wts[1]
            ccs = {}
            for b in range(B):
                cc = qkvcp.tile([P, F], mmdt, tag=f"c{b}", name=f"c{b}")
                ccs[b] = cc
                for h2 in range(2):
                    ps = ppool.tile([P, 512], f32, tag="pj", name="pj")
                    for kc in range(8):
                        nc.tensor.matmul(ps, xt_all[:, kc, 128 * b:128 * (b + 1)],
                                         wt[kc][:, 512 * h2:512 * (h2 + 1)],
                                         start=(kc == 0), stop=False)
                    nc.tensor.matmul(ps, ones_all[0:1, :],
                                     bv_sb[0:1, 512 * h2:512 * (h2 + 1)],
                                     start=False, stop=True)
                    nc.vector.tensor_copy(cc[:, 512 * h2:512 * (h2 + 1)], ps)
            # remaining dep-free str DMAs BEFORE the dependent vs/V2 DMAs
            load_dma(2)
            load_dma(3)
            vs = {}
            for b in range(B):
                vs[b] = dramp.tile([P, F], mmdt, tag=f"vs{b}", name=f"vs{b}")
                nc.sync.dma_start(out=vs[b], in_=ccs[b][:])
            for b in range(B):
                for hp in range(HPC):
                    nc.gpsimd.memset(V2[b, hp], 0.0)
                    dcol = 64 * hp
                    ones_col = 64 if hp == 0 else 0
                    src = vs[b][64 * hp:64 * hp + 64, :].rearrange(
                        "(j r) (cb d) -> (r cb) j d", j=8, cb=16)
                    nc.sync.dma_start(out=V2[b, hp][:, :, dcol:dcol + 64], in_=src)
                    nc.gpsimd.memset(V2[b, hp][:, :, ones_col:ones_col + 1], 1.0)
            load_exp(1)

            proj_qk(1, KT_all)

        # ---------- phase 2+3: attention + output projection, per batch ----------
        with tc.tile_pool(name="ep", bufs=4) as epool, \
                tc.tile_pool(name="esc", bufs=4) as escp, \
                tc.tile_pool(name="misc", bufs=2) as miscp, \
                tc.tile_pool(name="os", bufs=4) as osp, \
                tc.tile_pool(name="aps", bufs=1, space="PSUM") as aps, \
                tc.tile_pool(name="qkps", bufs=2, space="PSUM") as qkps:
            state = {b: {"qks": {}, "rbc": {}} for b in range(B)}

            def do_qk(b, hp, j):
                base = 64 * hp
                qq = qkps.tile([P, S], f32, tag="qk", name="qk")
                for h2 in range(2):
                    nc.tensor.matmul(
                        qq[:, 512 * h2:512 * (h2 + 1)],
                        KT_all[base:base + 64, S * b + 128 * j:S * b + 128 * (j + 1)],
                        QT_all[base:base + 64, S * b + 512 * h2:S * b + 512 * (h2 + 1)],
                        start=True, stop=True)
                state[b]["qks"][j] = qq

            def att_front(b):
                # rowsums + 1/rowsum broadcast + in-place eM normalize, both heads
                eM = eMslots[b]
                rbc = state[b]["rbc"]
                ps_r1 = aps.tile([P, 512], f32, tag="r1", name="r1")
                r1sb = miscp.tile([P, 512], mmdt, tag="r1sb", name="r1sb")
                for hp in range(HPC):
                    for j in range(8):
                        jl = 128 * j if causal else 0
                        t = eM[hp, j]
                        for h2 in range(2):
                            lo = max(512 * h2, jl)
                            hi = 512 * (h2 + 1)
                            if lo < hi:
                                last_j = (3 if h2 == 0 else 7) if causal else 7
                                r = R1ROW[hp, h2]
                                nc.tensor.matmul(
                                    ps_r1[r:r + 1, lo - 512 * h2:hi - 512 * h2],
                                    ones_all[:, 0:1], t[:, lo - jl:hi - jl],
                                    start=(j == 0), stop=(j == last_j))
                    for h2 in range(2):
                        r = R1ROW[hp, h2]
                        nc.vector.reciprocal(r1sb[r:r + 1, :], ps_r1[r:r + 1, :])
                    rb = miscp.tile([P, S], mmdt, tag=f"r1bc{hp}", name=f"r1bc{hp}")
                    rbc[hp] = rb
                    for h2 in range(2):
                        sl = slice(512 * h2, 512 * (h2 + 1))
                        r = R1ROW[hp, h2]
                        psb = aps.tile([P, 512], f32, tag="bc", name="bc")
                        nc.tensor.matmul(psb, ones_all[r:r + 1, :], r1sb[r:r + 1, :],
                                         start=True, stop=True)
                        nc.vector.tensor_copy(rb[:, sl], psb)
                    for j in range(8):
                        jl = 128 * j if causal else 0
                        nc.gpsimd.tensor_mul(eM[hp, j], eM[hp, j], rb[:, jl:])
                # early QK for head 0: ready the moment att_back(b) starts
                do_qk(b, 0, 0)
                do_qk(b, 0, 1)

            def att_back(b):
                eM = eMslots[b]
                rbc = state[b]["rbc"]
                qks = state[b]["qks"]
                for hp in range(HPC):
                    pv = [aps.tile([P, 512], f32, tag=f"pv{h2}", name=f"pv{h2}")
                          for h2 in range(2)]
                    for j in range(8):
                        jl = 128 * j if causal else 0
                        w = S - jl
                        Ej = epool.tile([P, S], mmdt, tag="E", name="E")
                        qq = qks.pop(j)
                        # masked region: E = exp(qk)
                        if jl > 0:
                            nc.scalar.activation(Ej[:, 0:jl], qq[:, 0:jl], Exp)
                        # unmasked: esc = sm + qk (cols shifted by jl), one exp
                        esc = escp.tile([P, S], f32, tag="esc", name="esc")
                        nc.vector.tensor_add(esc[:, 0:w], eM[hp, j][:, 0:w], qq[:, jl:])
                        nc.scalar.activation(Ej[:, jl:], esc[:, 0:w], Exp)
                        if j + 2 < 8:
                            do_qk(b, hp, j + 2)
                        for h2 in range(2):
                            nc.tensor.matmul(pv[h2], V2[b, hp][:, j, :],
                                             Ej[:, 512 * h2:512 * (h2 + 1)],
                                             start=(j == 0), stop=(j == 7))
                    if hp == 0:
                        # next head's first QKs ahead of the r2 chain
                        do_qk(b, 1, 0)
                        do_qk(b, 1, 1)

                    # normalize rows of PV by 1/rowsum2 (from the ones column)
                    sum_row = 64 if hp == 0 else 0
                    dlo = 64 * hp
                    r2sb = miscp.tile([P, S], mmdt, tag="r2sb", name="r2sb")
                    r2bc = miscp.tile([P, S], mmdt, tag="r2bc", name="r2bc")
                    for h2 in range(2):
                        sl = slice(512 * h2, 512 * (h2 + 1))
                        nc.vector.reciprocal(r2sb[sum_row:sum_row + 1, sl],
                                             pv[h2][sum_row:sum_row + 1, :])
                        psb = aps.tile([P, 512], f32, tag="bc", name="bc")
                        nc.tensor.matmul(psb[dlo:dlo + 64, :],
                                         ones_all[sum_row:sum_row + 1, 0:64],
                                         r2sb[sum_row:sum_row + 1, sl],
                                         start=True, stop=True)
                        nc.scalar.copy(r2bc[dlo:dlo + 64, sl], psb[dlo:dlo + 64, :])
                        nc.vector.tensor_mul(OT[b][dlo:dlo + 64, sl], pv[h2][dlo:dlo + 64, :],
                                             r2bc[dlo:dlo + 64, sl])

            def outproj(b):
                # psum rotates over pv0/pv1/bc (NOT qk: those hold the early QKs
                # of the next batch, already issued by att_front(b+1))
                ptags = [(aps, "pv0"), (aps, "pv1"), (aps, "bc")]
                for fo in range(8):
                    ot = osp.tile([P, S], mmdt, tag="os", name="os")
                    for h2 in range(2):
                        pool, tg = ptags[(2 * fo + h2) % 3]
                        ps = pool.tile([P, 512], f32, tag=tg, name="op")
                        nc.tensor.matmul(ps, wo_sb[:, 128 * fo:128 * (fo + 1)],
                                         OT[b][:, 512 * h2:512 * (h2 + 1)],
                                         start=True, stop=True)
                        eng = nc.scalar.copy if h2 == 0 else nc.vector.tensor_copy
                        eng(ot[:, 512 * h2:512 * (h2 + 1)], ps)
                    nc.sync.dma_start(out=out_d[b, 128 * fo:128 * (fo + 1), :], in_=ot)

            att_front(0)
            for b in range(B):
                att_back(b)
                if b + 2 < B:
                    load_exp(b + 2, hps=(0,))
                if b + 1 < B:
                    att_front(b + 1)
                if b + 2 < B:
                    load_exp(b + 2, hps=(1,))
                outproj(b)

    nc.compile()
    return nc


def _np_mmdt():
    from concourse import mybir
    return mybir.dt.np(getattr(mybir.dt, MM_DT))


def _prep_host(x, str_mat, attn_mask, Wq, bq, Wk, bk, Wv, bv, Wo, bo):
    npdt = _np_mmdt()
    x = np.asarray(x, np.float32)
    str_mat = np.asarray(str_mat, np.float32)
    attn_mask = np.asarray(attn_mask, np.float32)
    mask = attn_mask[:, 0]  # [b, s, s]
    causal = bool((mask == np.tril(np.ones((S, S), np.float32))[None]).all())
    strT = np.where(mask[:, None] == 0.0, NEG_FILL, str_mat).transpose(0, 1, 3, 2)
    strT = strT.astype(npdt)
    # xt layout [p (f within chunk), kc, (b, r)] per core, prepacked on host
    # x[b, s, f]: s = 128c + r (core c), f = kc*128 + p
    xt = x.reshape(B, 8, P, 8, P)            # [b, c, r, kc, p]
    xt = xt.transpose(4, 3, 0, 2, 1)         # [p, kc, b, r, c]
    xt = np.ascontiguousarray(xt).astype(npdt)  # [128, 8, 4, 128, 8]
    Wq_s = (np.asarray(Wq, np.float32) / D).astype(npdt)
    bq_s = np.asarray(bq, np.float32) / D
    bk_f = np.asarray(bk, np.float32)
    # bqkT[d + 64*hp, 16*t + cb] = b_t[64*cb + d]  (duplicated across hp halves)
    bqkT = np.empty((P, 32), np.float32)
    for t_i, bvec in enumerate((bq_s, bk_f)):
        blk = bvec.reshape(16, 64).T  # [d, cb]
        bqkT[0:64, 16 * t_i:16 * t_i + 16] = blk
        bqkT[64:128, 16 * t_i:16 * t_i + 16] = blk
    bv_c = np.asarray(bv, np.float32).reshape(1, F).astype(npdt)
    Wk_c = np.asarray(Wk, np.float32).astype(npdt)
    Wv_c = np.asarray(Wv, np.float32).astype(npdt)
    Wo_c = np.asarray(Wo, np.float32).astype(npdt)
    in_maps = []
    for c in range(NCORES):
        in_maps.append({
            "xT": np.ascontiguousarray(xt[:, :, :, :, c].reshape(P, 8, 4 * P)),
            "strT": np.ascontiguousarray(strT[:, HPC * c:HPC * (c + 1)]),
            "wq": Wq_s, "wk": Wk_c, "wv": Wv_c,
            "wo": np.ascontiguousarray(Wo_c[P * c:P * (c + 1)]),
            "bqkT": bqkT, "bv": bv_c,
        })
    return in_maps, causal


def _finish_host(partials, inputs):
    out = np.sum(np.asarray(partials, np.float32), axis=0, dtype=np.float32)  # [b, f, s]
    out = out.transpose(0, 2, 1) + np.asarray(inputs["bo"], np.float32)
    return np.ascontiguousarray(out.astype(np.float32))


def kernel(**inputs):
    from concourse.bass_utils import run_bass_kernel_spmd

    in_maps, causal = _prep_host(**inputs)
    key = (MM_DT, causal)
    if key not in _CACHE:
        _CACHE[key] = _build_nc(mm_dt_name=key[0], causal=causal)
    nc = _CACHE[key]
    res = run_bass_kernel_spmd(nc, in_maps, core_ids=list(range(NCORES)))
    partials = np.stack([np.asarray(r["outT"], np.float32) for r in res.results])
    return _finish_host(partials, inputs)


# revision 41
# speedup vs baseline: 1.3596x; 1.3596x over previous
"""Trainium2 Bass kernel for nn_MHInrAttn (sparse_attention, b=4 s=1024 f=1024 h=16).

Strategy (8 NeuronCores):
  - The reference uses a raw .reshape(b, h, s, d_h) with NO transpose, so head h's
    Q/K/V data comes from ROWS [64h, 64h+64) of the projected [s, f] matrix.
    Sharding 2 heads per core means each core only needs 128 rows of x per batch.
  - Per core: project Q/K/V for its 128 rows (all 4 batches), run attention for its
    2 heads x 4 batches in a "transposed" orientation (scores^T [k, q]), and produce
    a partial output projection (its heads' contribution through Wo rows).
  - Host: shard inputs, run SPMD on 8 cores, sum the 8 partials, transpose, add bo.

Device-side details:
  - Q^T/K^T are produced directly by W^T @ x^T matmuls (batches packed in the
    moving dim); DVE copies shuffle psum into the [d + 64*hp, q'] layout with the
    projection bias folded in via tensor_scalar_add (no PE transposes).
  - str_mat is host-transposed+masked (-40 fill), sent bf16; exp runs on Act as
    soon as each chunk's DMA lands (hoisted ahead of the projection phase).
  - softmax1: rowsums via ones-column matmuls into one psum bank (rows 0/32/64),
    1/rowsum broadcast via K=1 matmuls; eM is then normalized in place (bf16 DVE).
  - PV matmul carries an extra ones column in V for softmax2 row sums.
  - All matmul operands are bf16 (1 cycle/row); psum accumulation and the
    exp-argument chain stay fp32. Output partials are written bf16 and summed
    in fp32 on the host.
"""

import numpy as np

B, S, F, H, D = 4, 1024, 1024, 16, 64
NCORES = 8
HPC = H // NCORES  # heads per core
P = 128
NEG_FILL = -40.0
MM_DT = "bfloat16"
# psum partition row per (hp, h2) r1 accumulation chain; matmul output base
# partitions must be 0/32/64, so the 4th chain reuses row 0 (freed by the
# hp=0 reciprocal long before, in program order).
R1ROW = {(0, 0): 0, (0, 1): 32, (1, 0): 64, (1, 1): 0}

_CACHE = {}


def _build_nc(mm_dt_name="bfloat16", causal=True):
    from contextlib import ExitStack

    import concourse.bacc as bacc
    import concourse.tile as tile
    from concourse import mybir

    dt = mybir.dt
    f32 = dt.float32
    mmdt = getattr(dt, mm_dt_name)
    Exp = mybir.ActivationFunctionType.Exp
    Identity = mybir.ActivationFunctionType.Identity

    nc = bacc.Bacc("TRN2", target_bir_lowering=False, debug=False)

    xT_d = nc.dram_tensor("xT", [P, 8, 4 * P], mmdt, kind="ExternalInput").ap()
    str_d = nc.dram_tensor("strT", [B, HPC, S, S], mmdt, kind="ExternalInput").ap()
    wq_d = nc.dram_tensor("wq", [F, F], mmdt, kind="ExternalInput").ap()
    wk_d = nc.dram_tensor("wk", [F, F], mmdt, kind="ExternalInput").ap()
    wv_d = nc.dram_tensor("wv", [F, F], mmdt, kind="ExternalInput").ap()
    wo_d = nc.dram_tensor("wo", [P, F], mmdt, kind="ExternalInput").ap()
    bqkT_d = nc.dram_tensor("bqkT", [P, 32], f32, kind="ExternalInput").ap()
    bv_d = nc.dram_tensor("bv", [1, F], mmdt, kind="ExternalInput").ap()
    out_d = nc.dram_tensor("outT", [B, F, S], mmdt, kind="ExternalOutput").ap()

    with ExitStack() as ctx:
        ctx.enter_context(nc.allow_low_precision(
            reason="bf16 operands; all matmul accumulation stays in f32 psum"))
        tc = ctx.enter_context(tile.TileContext(nc))
        consts = ctx.enter_context(tc.tile_pool(name="consts", bufs=1))
        qtkt = ctx.enter_context(tc.tile_pool(name="qtkt", bufs=1))
        v2p = ctx.enter_context(tc.tile_pool(name="v2", bufs=1))
        outp = ctx.enter_context(tc.tile_pool(name="outp", bufs=1))
        wop = ctx.enter_context(tc.tile_pool(name="wop", bufs=1))
        emp = ctx.enter_context(tc.tile_pool(name="em", bufs=4))
        dramp = ctx.enter_context(tc.tile_pool(name="dram", bufs=1, space="DRAM"))

        ones_all = consts.tile([P, P], mmdt, tag="ones", name="ones")
        nc.vector.memset(ones_all, 1.0)
        bqkT_sb = consts.tile([P, 32], f32, tag="bqkT", name="bqkT")
        nc.sync.dma_start(out=bqkT_sb, in_=bqkT_d)
        bv_sb = consts.tile([1, F], mmdt, tag="bv", name="bv")
        nc.sync.dma_start(out=bv_sb, in_=bv_d)
        wo_sb = wop.tile([P, F], mmdt, tag="wo", name="wo")
        nc.sync.dma_start(out=wo_sb, in_=wo_d)

        QT_all = qtkt.tile([P, B * S], mmdt, tag="qt", name="qt")
        KT_all = qtkt.tile([P, B * S], mmdt, tag="kt", name="kt")
        OT, V2 = {}, {}
        for b in range(B):
            OT[b] = outp.tile([P, S], mmdt, tag=f"ot{b}", name=f"ot{b}")
            for hp in range(HPC):
                V2[b, hp] = v2p.tile([P, 8, P], mmdt, tag=f"v{b}{hp}", name=f"v{b}{hp}")

        # str_mat loads: DMA (SP, dep-free) + exp (Act), all four batches resident
        eMslots = [dict() for _ in range(B)]

        def load_dma(b):
            slot = eMslots[b]
            for hp in range(HPC):
                for j in range(8):
                    jl = 128 * j if causal else 0
                    t = emp.tile([P, S - jl], mmdt, tag=f"e{hp}{j}", name=f"e{hp}{j}")
                    slot[hp, j] = t
                    nc.sync.dma_start(out=t, in_=str_d[b, hp, 128 * j:128 * (j + 1), jl:])

        def load_exp(b, hps=(0, 1)):
            slot = eMslots[b]
            for hp in hps:
                for j in range(8):
                    nc.scalar.activation(slot[hp, j], slot[hp, j], Exp)

        # ---------- phase 1: projections + layout shuffles ----------
        with tc.tile_pool(name="xt", bufs=1) as xtp, \
                tc.tile_pool(name="wpool", bufs=1) as wp, \
                tc.tile_pool(name="qkvc", bufs=1) as qkvcp, \
                tc.tile_pool(name="pj", bufs=4, space="PSUM") as ppool:
            xt_all = xtp.tile([P, 8, 4 * P], mmdt, tag="xt", name="xt")
            nc.sync.dma_start(out=xt_all, in_=xT_d)

            # all weight DMAs up front (distinct tags; no dependent DMA may
            # precede a dep-free one on the in-order SP queue)
            wts = {}
            for t_i, w_d in enumerate((wq_d, wv_d, wk_d)):
                wts[t_i] = []
                for i in range(8):
                    w_tile = wp.tile([P, F], mmdt, tag=f"w{t_i}{i}", name=f"w{t_i}{i}")
                    nc.sync.dma_start(out=w_tile, in_=w_d[i * P:(i + 1) * P, :])
                    wts[t_i].append(w_tile)
            load_dma(0)
            load_dma(1)

            def proj_qk(t_i, dstT):
                wt = wts[0 if t_i == 0 else 2]
                for i in range(8):
                    ps = ppool.tile([P, 512], f32, tag="pj", name="pj")
                    for kc in range(8):
                        nc.tensor.matmul(ps, wt[kc][:, 128 * i:128 * (i + 1)],
                                         xt_all[:, kc, :],
                                         start=(kc == 0), stop=(kc == 7))
                    ps4 = ps.rearrange("p (b h r) -> p b h r", b=4, h=2)
                    for ch in range(2):
                        cb = 2 * i + ch
                        for hp in range(2):
                            src = ps4[64 * ch:64 * ch + 64, :, hp, :]
                            dst = dstT[64 * hp:64 * hp + 64, :].rearrange(
                                "p (b r c) -> p b r c", b=4, c=16)[:, :, :, cb]
                            bias_ap = bqkT_sb[64 * hp:64 * hp + 64,
                                              16 * t_i + cb:16 * t_i + cb + 1]
                            nc.vector.tensor_scalar_add(dst, src, bias_ap)

            proj_qk(0, QT_all)
            load_exp(0)

            # V: x @ Wv orientation (rows on partitions), shuffle through DRAM
            wt = wts[1]
            ccs = {}
            for b in range(B):
                cc = qkvcp.tile([P, F], mmdt, tag=f"c{b}", name=f"c{b}")
                ccs[b] = cc
                for h2 in range(2):
                    ps = ppool.tile([P, 512], f32, tag="pj", name="pj")
                    for kc in range(8):
                        nc.tensor.matmul(ps, xt_all[:, kc, 128 * b:128 * (b + 1)],
                                         wt[kc][:, 512 * h2:512 * (h2 + 1)],
                                         start=(kc == 0), stop=False)
                    nc.tensor.matmul(ps, ones_all[0:1, :],
                                     bv_sb[0:1, 512 * h2:512 * (h2 + 1)],
                                     start=False, stop=True)
                    nc.vector.tensor_copy(cc[:, 512 * h2:512 * (h2 + 1)], ps)
            # remaining dep-free str DMAs BEFORE the dependent vs/V2 DMAs
            load_dma(2)
            load_dma(3)
            vs = {}
            for b in range(B):
                vs[b] = dramp.tile([P, F], mmdt, tag=f"vs{b}", name=f"vs{b}")
                nc.sync.dma_start(out=vs[b], in_=ccs[b][:])
            for b in range(B):
                for hp in range(HPC):
                    nc.gpsimd.memset(V2[b, hp], 0.0)
                    dcol = 64 * hp
                    ones_col = 64 if hp == 0 else 0
                    src = vs[b][64 * hp:64 * hp + 64, :].rearrange(
                        "(j r) (cb d) -> (r cb) j d", j=8, cb=16)
                    nc.sync.dma_start(out=V2[b, hp][:, :, dcol:dcol + 64], in_=src)
                    nc.gpsimd.memset(V2[b, hp][:, :, ones_col:ones_col + 1], 1.0)
            load_exp(1)

            proj_qk(1, KT_all)

        # ---------- phase 2+3: attention + output projection, per batch ----------
        with tc.tile_pool(name="ep", bufs=6) as epool, \
                tc.tile_pool(name="esc", bufs=6) as escp, \
                tc.tile_pool(name="misc", bufs=2) as miscp, \
                tc.tile_pool(name="os", bufs=4) as osp, \
                tc.tile_pool(name="aps", bufs=1, space="PSUM") as aps, \
                tc.tile_pool(name="qkps", bufs=2, space="PSUM") as qkps:
            state = {b: {"qks": {}, "rbc": {}} for b in range(B)}

            def do_qk(b, hp, j):
                base = 64 * hp
                qq = qkps.tile([P, S], f32, tag="qk", name="qk")
                for h2 in range(2):
                    nc.tensor.matmul(
                        qq[:, 512 * h2:512 * (h2 + 1)],
                        KT_all[base:base + 64, S * b + 128 * j:S * b + 128 * (j + 1)],
                        QT_all[base:base + 64, S * b + 512 * h2:S * b + 512 * (h2 + 1)],
                        start=True, stop=True)
                state[b]["qks"][j] = qq

            def att_front(b):
                # rowsums + 1/rowsum broadcast + in-place eM normalize, both heads
                eM = eMslots[b]
                rbc = state[b]["rbc"]
                ps_r1 = aps.tile([P, 512], f32, tag="r1", name="r1")
                r1sb = miscp.tile([P, 512], mmdt, tag="r1sb", name="r1sb")
                for hp in range(HPC):
                    for j in range(8):
                        jl = 128 * j if causal else 0
                        t = eM[hp, j]
                        for h2 in range(2):
                            lo = max(512 * h2, jl)
                            hi = 512 * (h2 + 1)
                            if lo < hi:
                                last_j = (3 if h2 == 0 else 7) if causal else 7
                                r = R1ROW[hp, h2]
                                nc.tensor.matmul(
                                    ps_r1[r:r + 1, lo - 512 * h2:hi - 512 * h2],
                                    ones_all[:, 0:1], t[:, lo - jl:hi - jl],
                                    start=(j == 0), stop=(j == last_j))
                    for h2 in range(2):
                        r = R1ROW[hp, h2]
                        nc.vector.reciprocal(r1sb[r:r + 1, :], ps_r1[r:r + 1, :])
                    rb = miscp.tile([P, S], mmdt, tag=f"r1bc{hp}", name=f"r1bc{hp}")
                    rbc[hp] = rb
                    for h2 in range(2):
                        sl = slice(512 * h2, 512 * (h2 + 1))
                        r = R1ROW[hp, h2]
                        psb = aps.tile([P, 512], f32, tag="bc", name="bc")
                        nc.tensor.matmul(psb, ones_all[r:r + 1, :], r1sb[r:r + 1, :],
                                         start=True, stop=True)
                        nc.vector.tensor_copy(rb[:, sl], psb)
                    for j in range(8):
                        jl = 128 * j if causal else 0
                        nc.gpsimd.tensor_mul(eM[hp, j], eM[hp, j], rb[:, jl:])
                # early QK for head 0: ready the moment att_back(b) starts
                do_qk(b, 0, 0)
                do_qk(b, 0, 1)

            def att_back(b, filler=()):
                eM = eMslots[b]
                rbc = state[b]["rbc"]
                qks = state[b]["qks"]
                filler = list(filler)
                for hp in range(HPC):
                    pv = [aps.tile([P, 512], f32, tag=f"pv{h2}", name=f"pv{h2}")
                          for h2 in range(2)]
                    for j in range(8):
                        jl = 128 * j if causal else 0
                        w = S - jl
                        Ej = epool.tile([P, S], mmdt, tag="E", name="E")
                        qq = qks.pop(j)
                        # masked region: E = exp(qk)
                        if jl > 0:
                            nc.scalar.activation(Ej[:, 0:jl], qq[:, 0:jl], Exp)
                        # unmasked: esc = sm + qk (cols shifted by jl), one exp
                        esc = escp.tile([P, S], f32, tag="esc", name="esc")
                        nc.vector.tensor_add(esc[:, 0:w], eM[hp, j][:, 0:w], qq[:, jl:])
                        nc.scalar.activation(Ej[:, jl:], esc[:, 0:w], Exp)
                        if j + 2 < 8:
                            do_qk(b, hp, j + 2)
                        for h2 in range(2):
                            nc.tensor.matmul(pv[h2], V2[b, hp][:, j, :],
                                             Ej[:, 512 * h2:512 * (h2 + 1)],
                                             start=(j == 0), stop=(j == 7))
                        if hp == 0 and filler:
                            filler.pop(0)()
                    if hp == 0:
                        # next head's first QKs ahead of the r2 chain
                        do_qk(b, 1, 0)
                        do_qk(b, 1, 1)

                    # normalize rows of PV by 1/rowsum2 (from the ones column)
                    sum_row = 64 if hp == 0 else 0
                    dlo = 64 * hp
                    r2sb = miscp.tile([P, S], mmdt, tag="r2sb", name="r2sb")
                    r2bc = miscp.tile([P, S], mmdt, tag="r2bc", name="r2bc")
                    for h2 in range(2):
                        sl = slice(512 * h2, 512 * (h2 + 1))
                        nc.vector.reciprocal(r2sb[sum_row:sum_row + 1, sl],
                                             pv[h2][sum_row:sum_row + 1, :])
                        psb = aps.tile([P, 512], f32, tag="bc", name="bc")
                        nc.tensor.matmul(psb[dlo:dlo + 64, :],
                                         ones_all[sum_row:sum_row + 1, 0:64],
                                         r2sb[sum_row:sum_row + 1, sl],
                                         start=True, stop=True)
                        nc.scalar.copy(r2bc[dlo:dlo + 64, sl], psb[dlo:dlo + 64, :])
                        nc.vector.tensor_mul(OT[b][dlo:dlo + 64, sl], pv[h2][dlo:dlo + 64, :],
                                             r2bc[dlo:dlo + 64, sl])

            def outproj_piece(b, fo, ptags):
                ot = osp.tile([P, S], mmdt, tag="os", name="os")
                for h2 in range(2):
                    pool, tg = ptags[(2 * fo + h2) % len(ptags)]
                    ps = pool.tile([P, 512], f32, tag=tg, name="op")
                    nc.tensor.matmul(ps, wo_sb[:, 128 * fo:128 * (fo + 1)],
                                     OT[b][:, 512 * h2:512 * (h2 + 1)],
                                     start=True, stop=True)
                    eng = nc.scalar.copy if h2 == 0 else nc.vector.tensor_copy
                    eng(ot[:, 512 * h2:512 * (h2 + 1)], ps)
                nc.sync.dma_start(out=out_d[b, 128 * fo:128 * (fo + 1), :], in_=ot)

            def outproj_pieces(b):
                # interleaved into att_back(b+1) hp0: use the bc/r1 banks,
                # idle there (pv/qk hold the live accumulators)
                ptags = [(aps, "bc"), (aps, "r1")]
                return [lambda fo=fo: outproj_piece(b, fo, ptags) for fo in range(8)]

            blk_tags = [(aps, "pv0"), (aps, "pv1"), (aps, "bc")]
            att_front(0)
            for b in range(B):
                att_back(b)
                if b + 2 < B:
                    load_exp(b + 2, hps=(0,))
                if b + 1 < B:
                    att_front(b + 1)
                if b + 2 < B:
                    load_exp(b + 2, hps=(1,))
                for fo in range(8):
                    outproj_piece(b, fo, blk_tags)

    nc.compile()
    return nc


def _np_mmdt():
    from concourse import mybir
    return mybir.dt.np(getattr(mybir.dt, MM_DT))


def _prep_host(x, str_mat, attn_mask, Wq, bq, Wk, bk, Wv, bv, Wo, bo):
    npdt = _np_mmdt()
    x = np.asarray(x, np.float32)
    str_mat = np.asarray(str_mat, np.float32)
    attn_mask = np.asarray(attn_mask, np.float32)
    mask = attn_mask[:, 0]  # [b, s, s]
    causal = bool((mask == np.tril(np.ones((S, S), np.float32))[None]).all())
    strT = np.where(mask[:, None] == 0.0, NEG_FILL, str_mat).transpose(0, 1, 3, 2)
    strT = strT.astype(npdt)
    # xt layout [p (f within chunk), kc, (b, r)] per core, prepacked on host
    # x[b, s, f]: s = 128c + r (core c), f = kc*128 + p
    xt = x.reshape(B, 8, P, 8, P)            # [b, c, r, kc, p]
    xt = xt.transpose(4, 3, 0, 2, 1)         # [p, kc, b, r, c]
    xt = np.ascontiguousarray(xt).astype(npdt)  # [128, 8, 4, 128, 8]
    Wq_s = (np.asarray(Wq, np.float32) / D).astype(npdt)
    bq_s = np.asarray(bq, np.float32) / D
    bk_f = np.asarray(bk, np.float32)
    # bqkT[d + 64*hp, 16*t + cb] = b_t[64*cb + d]  (duplicated across hp halves)
    bqkT = np.empty((P, 32), np.float32)
    for t_i, bvec in enumerate((bq_s, bk_f)):
        blk = bvec.reshape(16, 64).T  # [d, cb]
        bqkT[0:64, 16 * t_i:16 * t_i + 16] = blk
        bqkT[64:128, 16 * t_i:16 * t_i + 16] = blk
    bv_c = np.asarray(bv, np.float32).reshape(1, F).astype(npdt)
    Wk_c = np.asarray(Wk, np.float32).astype(npdt)
    Wv_c = np.asarray(Wv, np.float32).astype(npdt)
    Wo_c = np.asarray(Wo, np.float32).astype(npdt)
    in_maps = []
    for c in range(NCORES):
        in_maps.append({
            "xT": np.ascontiguousarray(xt[:, :, :, :, c].reshape(P, 8, 4 * P)),
            "strT": np.ascontiguousarray(strT[:, HPC * c:HPC * (c + 1)]),
            "wq": Wq_s, "wk": Wk_c, "wv": Wv_c,
            "wo": np.ascontiguousarray(Wo_c[P * c:P * (c + 1)]),
            "bqkT": bqkT, "bv": bv_c,
        })
    return in_maps, causal


def _finish_host(partials, inputs):
    out = np.sum(np.asarray(partials, np.float32), axis=0, dtype=np.float32)  # [b, f, s]
    out = out.transpose(0, 2, 1) + np.asarray(inputs["bo"], np.float32)
    return np.ascontiguousarray(out.astype(np.float32))


def kernel(**inputs):
    from concourse.bass_utils import run_bass_kernel_spmd

    in_maps, causal = _prep_host(**inputs)
    key = (MM_DT, causal)
    if key not in _CACHE:
        _CACHE[key] = _build_nc(mm_dt_name=key[0], causal=causal)
    nc = _CACHE[key]
    res = run_bass_kernel_spmd(nc, in_maps, core_ids=list(range(NCORES)))
    partials = np.stack([np.asarray(r["outT"], np.float32) for r in res.results])
    return _finish_host(partials, inputs)
